# revision 10
# baseline (speedup 1.0000x reference)
"""GRU kernel for Trainium2, 8 NeuronCores.

Strategy (feature-parallel, NOT the batch-parallel hint):
  - The T=1024 sequential scan is the critical path; each step must stream the
    3 recurrent weight matrices (12.6 MB fp32) through a PE array.  Sharding
    the 3*D=3072 gate output features across the 8 cores cuts that stream 8x.
  - Core c owns features [128c, 128(c+1)) of each gate (r, h, z).  Everything
    on-chip is feature-major ([128 feat, 64 batch] tiles), so there are zero
    on-chip transposes; the host pre-transposes `inputs` once.
  - Per step, each core broadcasts its H_{t+1} shard ([128, 64]) to all 8
    cores with remote_dma_broadcast (SBUF->SBUF, ~1-2us) instead of a ncfw
    collective (~5us floor, not loopable).
  - The input projections X = inputs @ W_x* are folded into the step loop:
    while step t's H-exchange is in flight, the PE computes X(t+1) and
    accumulates it directly into step t+1's PSUM tiles.  No X round-trip
    through DRAM, and the exchange latency is hidden behind PE work.
  - Matmuls run in bf16 (fp32 PSUM accumulation, fp32 hidden-state carry and
    elementwise math).  MM_BF16=False switches everything to fp32.

Gate math per step (reference semantics):
  R = sigmoid(Xr + H @ W_hr + b_r)
  Z = sigmoid(Xz + H @ W_hz + b_z)
  Ht = tanh(Xh + b_h + R * (H @ W_hh))
  H' = Z*H + (1-Z)*Ht = Ht + Z*(H - Ht)
"""

import numpy as np
import ml_dtypes

import concourse.bass as bass
from concourse import bacc, mybir
from concourse.bass import ds

T_TOTAL = 1024
B = 64
D = 1024
NCORES = 8
DSH = 128          # features per core per gate
KT = 8             # k-tiles (contraction 1024 = 8 * 128)
RIN = 4            # input-slice prefetch ring depth
MM_BF16 = True     # matmul operand dtype (PSUM accum always fp32)

_built = {}


def _build(T: int, mm_bf16: bool):
    dt_mm = mybir.dt.bfloat16 if mm_bf16 else mybir.dt.float32
    f32 = mybir.dt.float32

    nc = bacc.Bacc("TRN2", target_bir_lowering=False, debug=False,
                   num_devices=NCORES)

    # ---------------- DRAM ----------------
    inT = nc.dram_tensor("inT", (D, T * B), dt_mm, kind="ExternalInput")
    Wh = nc.dram_tensor("Wh", (D, 3 * DSH), dt_mm, kind="ExternalInput")
    Wx = nc.dram_tensor("Wx", (D, 3 * DSH), dt_mm, kind="ExternalInput")
    bias = nc.dram_tensor("bias", (3 * DSH,), f32, kind="ExternalInput")
    out = nc.dram_tensor("out", (T, DSH, B), f32, kind="ExternalOutput")

    # ---------------- SBUF ----------------
    Wh_sb = nc.alloc_sbuf_tensor("Wh_sb", [128, KT * 3 * DSH], dt_mm)
    Wx_sb = nc.alloc_sbuf_tensor("Wx_sb", [128, KT * 3 * DSH], dt_mm)
    in_ring = nc.alloc_sbuf_tensor("in_ring", [128, RIN * KT * B], dt_mm)
    Hx = nc.alloc_sbuf_tensor("Hx", [128, 2 * KT * B], dt_mm)   # gather buf
    Hsend = nc.alloc_sbuf_tensor("Hsend", [128, 2 * B], dt_mm)  # bcast src
    Hring = nc.alloc_sbuf_tensor("Hring", [128, 4 * B], f32)    # fp32 H
    bias_sb = nc.alloc_sbuf_tensor("bias_sb", [128, 3], f32)
    Rt = nc.alloc_sbuf_tensor("Rt", [128, B], f32)
    Zt = nc.alloc_sbuf_tensor("Zt", [128, B], f32)
    Ht = nc.alloc_sbuf_tensor("Ht", [128, B], f32)   # tanh output
    t1 = nc.alloc_sbuf_tensor("t1", [128, B], f32)
    t2 = nc.alloc_sbuf_tensor("t2", [128, B], f32)
    dd = nc.alloc_sbuf_tensor("dd", [128, B], f32)
    ee = nc.alloc_sbuf_tensor("ee", [128, B], f32)

    # ---------------- PSUM (one bank each; banks never shared) ----------
    # gen = step parity; 'hx' bank holds h (cols 0:64) and xh (cols 64:128)
    ps_r = [nc.alloc_psum_tensor(f"ps_r{g}", [128, B], f32) for g in range(2)]
    ps_z = [nc.alloc_psum_tensor(f"ps_z{g}", [128, B], f32) for g in range(2)]
    ps_hx = [nc.alloc_psum_tensor(f"ps_hx{g}", [128, 2 * B], f32)
             for g in range(2)]

    def wh_tile(j, g):
        return Wh_sb.ap()[:, j * 3 * DSH + g * DSH: j * 3 * DSH + (g + 1) * DSH]

    def wx_tile(j, g):
        return Wx_sb.ap()[:, j * 3 * DSH + g * DSH: j * 3 * DSH + (g + 1) * DSH]

    def hx_tile(par, j):
        return Hx.ap()[:, par * KT * B + j * B: par * KT * B + (j + 1) * B]

    def in_tile(s, j):
        o = (s % RIN) * KT * B + j * B
        return in_ring.ap()[:, o: o + B]

    def hring(s):
        return Hring.ap()[:, (s % 4) * B: (s % 4) * B + B]

    def hsend(par):
        return Hsend.ap()[:, par * B: (par + 1) * B]

    with (
        nc.semaphore() as dma_w_sem,
        nc.semaphore() as dma_x_sem,
        nc.semaphore() as dout_sem,
        nc.semaphore() as init_sem,
        nc.semaphore() as pe_r_sem,
        nc.semaphore() as pe_h_sem,
        nc.semaphore() as pe_z_sem,
        nc.semaphore() as pe_x_sem,
        nc.semaphore() as act_r_sem,
        nc.semaphore() as act_ht_sem,
        nc.semaphore() as act_z_sem,
        nc.semaphore() as act_cast_sem,
        nc.semaphore() as dve_t2_sem,
        nc.semaphore() as dve_hnew_sem,
        nc.semaphore() as prep_sem,
        nc.semaphore() as rsem,
        nc.semaphore() as lsem,
        nc.Block() as block,
    ):
        # ---------------- SP: all DMA ----------------
        @block.sync
        def _(sp):
            # weights / bias once
            sp.dma_start(
                Wh_sb.ap().rearrange("p (j m) -> p j m", j=KT),
                Wh.ap().rearrange("(j p) m -> p j m", p=128),
            ).then_inc(dma_w_sem, 16)
            sp.dma_start(
                Wx_sb.ap().rearrange("p (j m) -> p j m", j=KT),
                Wx.ap().rearrange("(j p) m -> p j m", p=128),
            ).then_inc(dma_w_sem, 16)
            with nc.allow_non_contiguous_dma(reason="one-time 384-elem bias"):
                sp.dma_start(
                    bias_sb.ap(),
                    bias.ap().rearrange("(c p) -> p c", p=128),
                ).then_inc(dma_w_sem, 16)
            # prime the input ring
            for s in range(min(RIN, T)):
                sp.dma_start(
                    in_ring.ap()[:, (s % RIN) * KT * B:(s % RIN + 1) * KT * B]
                    .rearrange("p (j b) -> p j b", j=KT),
                    inT.ap()[:, s * B:(s + 1) * B]
                    .rearrange("(j p) b -> p j b", p=128),
                ).then_inc(dma_x_sem, 16)
            for t in range(T):
                s = t + RIN
                if s < T:
                    # slot s%RIN last read by the X-group targeting step s-RIN
                    sp.wait_ge(pe_x_sem, s - RIN + 1)
                    sp.dma_start(
                        in_ring.ap()[:, (s % RIN) * KT * B:(s % RIN + 1) * KT * B]
                        .rearrange("p (j b) -> p j b", j=KT),
                        inT.ap()[:, s * B:(s + 1) * B]
                        .rearrange("(j p) b -> p j b", p=128),
                    ).then_inc(dma_x_sem, 16)
                # write out H_{t+1} (= outputs[t]) from Hring slot (t+1)%4
                sp.wait_ge(dve_hnew_sem, t + 1)
                sp.dma_start(
                    out.ap()[t], hring(t + 1)
                ).then_inc(dout_sem, 16)
            # teardown: all output DMAs must have landed in DRAM
            sp.wait_ge(dout_sem, 16 * T)

        # ---------------- PE ----------------
        @block.tensor
        def _(pe):
            pe.wait_ge(dma_w_sem, 48)
            pe.wait_ge(dma_x_sem, 16)
            # prologue: X(0) into gen-0 psums
            mm = None
            for g, tgt in ((0, ps_r[0].ap()), (1, ps_hx[0].ap()[:, B:2 * B]),
                           (2, ps_z[0].ap())):
                for j in range(KT):
                    mm = nc.tensor.matmul(tgt, wx_tile(j, g), in_tile(0, j),
                                          start=(j == 0), stop=(j == KT - 1),
                                          skip_group_check=True)
            mm.then_inc(pe_x_sem, 1)
            for t in range(T):
                gen = t % 2
                if t == 0:
                    pe.wait_ge(init_sem, 1)
                else:
                    pe.wait_ge(rsem, 16 * t)
                # H-part of the three gates; r first, then h, then z
                for g, tgt, sem, st in (
                    (0, ps_r[gen].ap(), pe_r_sem, False),
                    (1, ps_hx[gen].ap()[:, 0:B], pe_h_sem, True),
                    (2, ps_z[gen].ap(), pe_z_sem, False),
                ):
                    for j in range(KT):
                        mm = nc.tensor.matmul(tgt, wh_tile(j, g),
                                              hx_tile(gen, j),
                                              start=(st and j == 0),
                                              stop=(j == KT - 1),
                                              skip_group_check=True)
                    mm.then_inc(sem, 1)
                # X-part for step t+1 into the other generation
                if t + 1 < T:
                    pe.wait_ge(dma_x_sem, 16 * (t + 2))
                    if t >= 1:
                        # WAR: gen' psums were read by step t-1 consumers
                        pe.wait_ge(act_r_sem, t)
                        pe.wait_ge(act_z_sem, t)
                        pe.wait_ge(dve_t2_sem, t)
                    gp = (t + 1) % 2
                    for g, tgt in ((0, ps_r[gp].ap()),
                                   (1, ps_hx[gp].ap()[:, B:2 * B]),
                                   (2, ps_z[gp].ap())):
                        for j in range(KT):
                            mm = nc.tensor.matmul(tgt, wx_tile(j, g),
                                                  in_tile(t + 1, j),
                                                  start=(j == 0),
                                                  stop=(j == KT - 1),
                                                  skip_group_check=True)
                    mm.then_inc(pe_x_sem, 1)

        # ---------------- ACT ----------------
        @block.scalar
        def _(act):
            act.wait_ge(dma_w_sem, 48)
            for t in range(T):
                gen = t % 2
                act.wait_ge(pe_r_sem, t + 1)
                nc.scalar.activation(Rt.ap(), ps_r[gen].ap(),
                                     mybir.ActivationFunctionType.Sigmoid,
                                     bias=bias_sb.ap()[:, 0:1]
                                     ).then_inc(act_r_sem, 1)
                act.wait_ge(dve_t2_sem, t + 1)
                nc.scalar.activation(Ht.ap(), t2.ap(),
                                     mybir.ActivationFunctionType.Tanh,
                                     bias=bias_sb.ap()[:, 1:2]
                                     ).then_inc(act_ht_sem, 1)
                act.wait_ge(pe_z_sem, t + 1)
                nc.scalar.activation(Zt.ap(), ps_z[gen].ap(),
                                     mybir.ActivationFunctionType.Sigmoid,
                                     bias=bias_sb.ap()[:, 2:3]
                                     ).then_inc(act_z_sem, 1)
                if t + 1 < T:
                    # cast H_{t+1} for the broadcast
                    act.wait_ge(dve_hnew_sem, t + 1)
                    if t >= 2:
                        act.wait_ge(lsem, 16 * (t - 1))
                    nc.scalar.copy(hsend((t + 1) % 2),
                                   hring(t + 1)).then_inc(act_cast_sem, 1)

        # ---------------- DVE ----------------
        @block.vector
        def _(dve):
            # init: zero H state and the parity-0 gather buffer
            nc.vector.memset(Hring.ap()[:, 0:B], 0.0)
            nc.vector.memset(Hx.ap()[:, 0:KT * B], 0.0).then_inc(init_sem, 1)
            for t in range(T):
                gen = t % 2
                dve.wait_ge(pe_h_sem, t + 1)
                dve.wait_ge(act_r_sem, t + 1)
                nc.vector.tensor_mul(t1.ap(), Rt.ap(), ps_hx[gen].ap()[:, 0:B])
                nc.vector.tensor_add(t2.ap(), t1.ap(),
                                     ps_hx[gen].ap()[:, B:2 * B]
                                     ).then_inc(dve_t2_sem, 1)
                dve.wait_ge(act_ht_sem, t + 1)
                nc.vector.tensor_sub(dd.ap(), hring(t), Ht.ap())
                dve.wait_ge(act_z_sem, t + 1)
                nc.vector.tensor_mul(ee.ap(), Zt.ap(), dd.ap())
                if t >= 3:
                    dve.wait_ge(dout_sem, 16 * (t - 3))
                nc.vector.tensor_add(hring(t + 1), Ht.ap(),
                                     ee.ap()).then_inc(dve_hnew_sem, 1)

        # ---------------- GPSIMD: exchange ----------------
        @block.gpsimd
        def _(g):
            pid = g.partition_id()
            for t in range(T - 1):
                par = (t + 1) % 2
                inst = g.remote_dma_broadcast(
                    out_ap=Hx.ap()[:, ds(pid * B + par * (KT * B), B)],
                    in_ap=hsend(par),
                    remote_sem=rsem,
                    local_sem=lsem,
                    rdests=[(0, k) for k in range(NCORES)],
                )
                inst.then_inc(prep_sem, 1)
                g.wait_ge(prep_sem, t + 1)
                g.wait_ge(act_cast_sem, t + 1)
                g.trigger_dma(1)
            if T >= 2:
                g.wait_ge(lsem, 16 * (T - 1))

    nc.compile()
    return nc


def _prep_inputs(inputs, W_xz, W_hz, b_z, W_xr, W_hr, b_r, W_xh, W_hh, b_h,
                 mm_bf16):
    T = inputs.shape[0]
    np_mm = ml_dtypes.bfloat16 if mm_bf16 else np.float32
    # [T, B, D] -> [D, T*B]
    inT = np.ascontiguousarray(
        inputs.transpose(2, 0, 1).reshape(D, T * B)).astype(np_mm)
    in_maps = []
    for c in range(NCORES):
        sh = slice(c * DSH, (c + 1) * DSH)
        Wh_c = np.concatenate(
            [W_hr[:, sh], W_hh[:, sh], W_hz[:, sh]], axis=1).astype(np_mm)
        Wx_c = np.concatenate(
            [W_xr[:, sh], W_xh[:, sh], W_xz[:, sh]], axis=1).astype(np_mm)
        bias_c = np.concatenate(
            [b_r[sh], b_h[sh], b_z[sh]]).astype(np.float32)
        in_maps.append({
            "inT": inT,
            "Wh": np.ascontiguousarray(Wh_c),
            "Wx": np.ascontiguousarray(Wx_c),
            "bias": bias_c,
        })
    return in_maps


def _run(inputs, W_xz, W_hz, b_z, W_xr, W_hr, b_r, W_xh, W_hh, b_h,
         mm_bf16=MM_BF16, trace=False):
    from concourse.bass_utils import run_bass_kernel_spmd
    T = inputs.shape[0]
    key = (T, mm_bf16)
    if key not in _built:
        _built[key] = _build(T, mm_bf16)
    nc = _built[key]
    in_maps = _prep_inputs(inputs, W_xz, W_hz, b_z, W_xr, W_hr, b_r,
                           W_xh, W_hh, b_h, mm_bf16)
    res = run_bass_kernel_spmd(nc, in_maps, core_ids=list(range(NCORES)),
                               trace=trace)
    T_ = T
    outputs = np.empty((T_, B, D), np.float32)
    for c in range(NCORES):
        oc = res.results[c]["out"]          # [T, DSH, B] feature-major
        outputs[:, :, c * DSH:(c + 1) * DSH] = oc.transpose(0, 2, 1)
    return outputs, res


def kernel(inputs, W_xz, W_hz, b_z, W_xr, W_hr, b_r, W_xh, W_hh, b_h):
    inputs = np.asarray(inputs, np.float32)
    args = [np.asarray(a, np.float32) for a in
            (W_xz, W_hz, b_z, W_xr, W_hr, b_r, W_xh, W_hh, b_h)]
    outputs, _ = _run(inputs, *args, mm_bf16=MM_BF16)
    return outputs, outputs[-1].copy()
